# revision 41
# baseline (speedup 1.0000x reference)
"""Multi-head attention (B=4, N=2048, C=1024, H=16) on 8 Trainium2 NeuronCores.

Sharding: data parallel over batch (4-way) x tensor parallel over heads
(2-way, 8 heads per group). Core c handles batch c//2 and head group c%2.
Each core computes a partial projection output [2048, 1024]; the host sums
the two head-group partials per batch and adds b_proj.

v2 design (ACT-bound kernel; ScalarE exp is the bottleneck):
  - bf16 on all PE paths (x, weights, Q/K/V, P, O): halves SBUF, FWL weight
    loads; accuracy budget is rel<2e-2 vs fp32 reference.
  - QK scores via row-tiled matmul pairs (tile_position (64h,0)): each head
    of a pair uses its own 64 contraction rows; concurrent on HW.
  - exp split between ScalarE (activation Exp, [128,1024] chunks spanning
    both heads' PSUM banks) and VectorE (Schraudolph: one tensor_scalar
    fp32->int16 computing round((s*SCALE*log2e + 127-c)*128); the int16 bits
    ARE the bf16 pattern of ~exp(s*SCALE)). DVE handles DVE_KS of every 16
    k-chunks to offload the saturated ScalarE.
  - PV with [V|1] 65-wide bf16 stationary: the ones column accumulates the
    softmax denominators in PSUM row 64 for free.
  - normalization: PSUM->SBUF via DMA, reciprocal_approx_fast (DVE),
    denominator broadcast via DRAM round-trip DMA, final scale-mul on GpSimd
    writing bf16 O^T.
  - phases interleaved in emission order so the in-order engine queues keep
    ScalarE/VectorE saturated: prologue computes V and pair-0 Q/K; Q/K for
    pair hp+1 and the output projection for query-block qb are emitted
    between attention blocks.
"""

import numpy as np

B, N, C, H, D = 4, 2048, 1024, 16, 64
SCALE = float(D) ** -0.5
T = 2048          # tokens per core (one batch)
G = 2             # head groups
HL = H // G       # 8 local heads
NP = HL // 2      # 4 head pairs
F = HL * D        # 512 local features
VW = 66           # V tile column stride per head (64 data + 1 ones + 1 pad)
NCORES = 8

LOG2E = 1.4426950408889634
C_CORR = 0.04303566602587917
# DVE-exp chunk pattern within each 16-chunk block (tunable offload ratio)
DVE_KS = ()


def _build_bass(reps=1, dve_ks=DVE_KS, use_gpsimd=True, debug_taps=False):
    import concourse.bacc as bacc
    import concourse.tile as tile
    from concourse import mybir
    import concourse.bass as bass
    f32 = mybir.dt.float32
    bf16 = mybir.dt.bfloat16
    i16 = mybir.dt.int16
    EXP = mybir.ActivationFunctionType.Exp
    MULT = mybir.AluOpType.mult
    ADD = mybir.AluOpType.add

    A_MUL = float(SCALE * LOG2E * (1 << 7))
    B_ADD = float((127.0 - C_CORR) * (1 << 7))

    nc = bacc.Bacc("TRN2", debug=False, num_devices=NCORES)
    xt_d = nc.dram_tensor("xt", [C, T], bf16, kind="ExternalInput")
    wqkv_d = nc.dram_tensor("wqkv", [C, 3 * F], bf16, kind="ExternalInput")
    wproj_d = nc.dram_tensor("wproj", [F, C], bf16, kind="ExternalInput")
    ident_d = nc.dram_tensor("ident", [128, 128], bf16, kind="ExternalInput")
    y_d = nc.dram_tensor("y", [T, C], f32, kind="ExternalOutput")
    if debug_taps:
        qkt_dbg = nc.dram_tensor("qkt_dbg", [8 * 128, T], f32,
                                 kind="ExternalOutput")
        v_dbg = nc.dram_tensor("v_dbg", [16 * 128, HL * VW], f32,
                               kind="ExternalOutput")
        ot_dbg = nc.dram_tensor("ot_dbg", [NP * 128, T], f32,
                                kind="ExternalOutput")

    with tile.TileContext(nc) as tc:
        with (
            tc.tile_pool(name="xT", bufs=1) as xT_pool,
            tc.tile_pool(name="wv", bufs=1) as wv_pool,
            tc.tile_pool(name="wqk", bufs=1) as wqk_pool,
            tc.tile_pool(name="wp", bufs=1) as wp_pool,
            tc.tile_pool(name="qkt", bufs=1) as qkt_pool,
            tc.tile_pool(name="vsb", bufs=1) as v_pool,
            tc.tile_pool(name="ot", bufs=1) as ot_pool,
            tc.tile_pool(name="ptile", bufs=4) as p_pool,
            tc.tile_pool(name="ocp", bufs=6) as ocp_pool,
            tc.tile_pool(name="nrm", bufs=2) as nrm_pool,
            tc.tile_pool(name="ysb", bufs=2) as y_pool,
            tc.tile_pool(name="y01p", bufs=16) as y01_pool,
            tc.tile_pool(name="consts", bufs=1) as consts,
            tc.tile_pool(name="psqk", bufs=2, space="PSUM") as ps_qk,
            tc.tile_pool(name="psot", bufs=1, space="PSUM") as ps_ot,
            tc.tile_pool(name="psproj", bufs=2, space="PSUM") as ps_proj,
        ):
            IDENT = consts.tile([128, 128], bf16, name="IDENT")
            nc.sync.dma_start(out=IDENT, in_=ident_d.ap())
            ZW = consts.tile([128, 128], bf16, name="ZW")
            nc.vector.memset(ZW, 0.0)
            for _rep in range(reps):
                XT = [xT_pool.tile([128, T], bf16, tag=f"xT{c}", name=f"xT{c}")
                      for c in range(8)]
                WV = [wv_pool.tile([128, F], bf16, tag=f"wv{c}", name=f"wv{c}")
                      for c in range(8)]
                WQK = [wqk_pool.tile([128, 2 * F], bf16, tag=f"wqk{c}",
                                     name=f"wqk{c}") for c in range(8)]
                WP = [wp_pool.tile([128, C], bf16, tag=f"wp{p}", name=f"wp{p}")
                      for p in range(NP)]
                QKT = [qkt_pool.tile([128, T], bf16, tag=f"qkt{m}",
                                     name=f"qkt{m}") for m in range(8)]
                V = [v_pool.tile([128, HL * VW], bf16, tag=f"v{t}",
                                 name=f"v{t}") for t in range(16)]
                OT = [ot_pool.tile([128, T], bf16, tag=f"ot{p}", name=f"ot{p}")
                      for p in range(NP)]

                # loads: WQK/XT interleaved (HWDGE, prologue-critical);
                # WV/WP on the idle gpsimd SWDGE path
                for c in range(8):
                    nc.sync.dma_start(
                        out=WQK[c],
                        in_=wqkv_d.ap()[c * 128:(c + 1) * 128, 0:2 * F])
                    nc.sync.dma_start(out=XT[c],
                                      in_=xt_d.ap()[c * 128:(c + 1) * 128, :])
                for c in range(8):
                    nc.sync.dma_start(
                        out=WV[c],
                        in_=wqkv_d.ap()[c * 128:(c + 1) * 128, 2 * F:3 * F])
                for p in range(NP):
                    nc.sync.dma_start(
                        out=WP[p],
                        in_=wproj_d.ap()[p * 128:(p + 1) * 128, :])

                # ---- projection work units (emitted interleaved) ----------
                def emit_p2_halves(m, n):
                    """Q/K proj for QKT[m] tokens [512n, 512n+512) as two
                    pullable half-units (4 contraction chunks each)."""
                    qp = ps_proj.tile([128, 512], f32, tag="proj", name="qp")

                    def half(lo):
                        for c in range(lo, lo + 4):
                            nc.tensor.matmul(
                                qp,
                                WQK[c][:, m * 128:(m + 1) * 128],
                                XT[c][:, n * 512:(n + 1) * 512],
                                start=(c == 0), stop=(c == 7),
                            )
                        if lo == 4:
                            nc.vector.tensor_copy(
                                QKT[m][:, n * 512:(n + 1) * 512], qp)
                    return [lambda: half(0), lambda: half(4)]

                def emit_p2(m, n):
                    for u in emit_p2_halves(m, n):
                        u()

                def emit_p3_halves(t):
                    """V proj for token chunk t as two pullable halves."""
                    vp = ps_proj.tile([128, F], f32, tag="proj", name="vp")

                    def half(lo):
                        for c in range(lo, lo + 4):
                            nc.tensor.matmul(
                                vp,
                                XT[c][:, t * 128:(t + 1) * 128],
                                WV[c],
                                start=(c == 0), stop=(c == 7),
                            )
                        if lo == 4:
                            v3 = V[t].rearrange("p (h w) -> p h w", w=VW)
                            nc.vector.memset(v3[:, :, 64:65], 1.0)
                            nc.vector.tensor_copy(
                                v3[:, :, 0:64],
                                vp.rearrange("p (h w) -> p h w", w=64))
                    return [lambda: half(0), lambda: half(4)]

                def emit_p5a(t, n):
                    """output proj partial hp0+hp1 -> bf16 staging."""
                    yp = ps_proj.tile([128, 512], f32, tag="proj", name="yp")
                    for hp in range(2):
                        nc.tensor.matmul(
                            yp,
                            OT[hp][:, t * 128:(t + 1) * 128],
                            WP[hp][:, n * 512:(n + 1) * 512],
                            start=(hp == 0), stop=(hp == 1),
                        )
                    if n == 0:
                        y01s[t] = y01_pool.tile([128, 1024], bf16,
                                                tag="y01", name="y01")
                    nc.vector.tensor_copy(
                        y01s[t][:, n * 512:(n + 1) * 512], yp)

                def emit_p5b(t, n):
                    """hp2+hp3 + staged partial -> y DMA (merged n-halves)."""
                    yp = ps_proj.tile([128, 512], f32, tag="proj", name="yp")
                    for hp in range(2, 4):
                        nc.tensor.matmul(
                            yp,
                            OT[hp][:, t * 128:(t + 1) * 128],
                            WP[hp][:, n * 512:(n + 1) * 512],
                            start=(hp == 2), stop=(hp == 3),
                        )
                    if n == 0:
                        yts[t] = y_pool.tile([128, 1024], f32, tag="yt",
                                             name="yt")
                    nc.vector.tensor_add(
                        yts[t][:, n * 512:(n + 1) * 512], yp,
                        y01s[t][:, n * 512:(n + 1) * 512])
                    if n == 1:
                        nc.sync.dma_start(
                            out=y_d.ap()[t * 128:(t + 1) * 128, :],
                            in_=yts[t])

                yts = [None] * 16
                y01s = [None] * 16

                # ---- attention block: head pair hp, query block qb --------
                # O accumulates query-major: bank j holds qc=2j,2j+1, each
                # [128q, 65(h0)|65(h1)] -> [128, 260]; then per-partition
                # reciprocal of the ones-column, tensor_scalar normalize to
                # bf16, PE-transpose back to feature-major OT.
                def emit_block(hp, qb, pull):
                    q0 = qb * 512
                    KT = QKT[NP + hp]
                    QT = QKT[hp]
                    O = [ps_ot.tile([128, 260], f32, tag=f"o{j}", name=f"o{j}")
                         for j in range(2)]
                    for j in range(2):
                        nc.tensor.matmul(
                            O[j], ZW, QKT[0][:, 0:260],
                            start=True, stop=False, skip_group_check=True)

                    def emit_pv(k, P):
                        for h in range(2):
                            lh = 2 * hp + h
                            for qc in range(4):
                                nc.tensor.matmul(
                                    O[qc // 2][:, 130 * (qc % 2) + 65 * h:
                                               130 * (qc % 2) + 65 * h + 65],
                                    P[:, 512 * h + 128 * qc:
                                      512 * h + 128 * qc + 128],
                                    V[k][:, lh * VW:lh * VW + 65],
                                    start=False, stop=(k == 15),
                                    skip_group_check=True,
                                )

                    prev = None
                    for k in range(16):
                        s = ps_qk.tile([128, 1024], f32, tag="s", name="s")
                        for h in range(2):
                            nc.tensor.matmul(
                                s[:, 512 * h:512 * h + 512],
                                KT[64 * h:64 * h + 64,
                                   k * 128:(k + 1) * 128],
                                QT[64 * h:64 * h + 64, q0:q0 + 512],
                                start=True, stop=True,
                            )
                        P = p_pool.tile([128, 1024], bf16, tag="P", name="P")
                        if k in dve_ks:
                            nc.vector.tensor_scalar(
                                out=P.bitcast(i16), in0=s,
                                scalar1=A_MUL, scalar2=B_ADD,
                                op0=MULT, op1=ADD)
                        else:
                            nc.scalar.activation(P, s, EXP, scale=SCALE)
                        if prev is not None:
                            emit_pv(k - 1, prev)
                        prev = P
                        pull(k)
                    emit_pv(15, prev)
                    osb = [None] * 4
                    for j in range(2):
                        dens = O[j].rearrange("p (g w) -> p g w", w=65)
                        dcp = nrm_pool.tile([128, 4], f32, tag="dcp",
                                            name="dcp")
                        nc.vector.tensor_copy(
                            dcp.rearrange("p (g w) -> p g w", w=1),
                            dens[:, :, 64:65])
                        rec = nrm_pool.tile([128, 4], f32, tag="rec",
                                            name="rec")
                        nc.vector.reciprocal_approx_fast(rec, dcp)
                        for c in range(2):
                            qc = 2 * j + c
                            ob = ocp_pool.tile([128, 128], bf16, tag="osb",
                                               name="osb")
                            for h in range(2):
                                nc.vector.tensor_scalar(
                                    out=ob[:, 64 * h:64 * h + 64],
                                    in0=O[j][:, 130 * c + 65 * h:
                                             130 * c + 65 * h + 64],
                                    scalar1=rec[:, 2 * c + h:2 * c + h + 1],
                                    scalar2=None, op0=MULT)
                            osb[qc] = ob
                    tp = ps_ot.tile([128, 512], bf16, tag="o0", name="tp")
                    for qc in range(4):
                        nc.tensor.transpose(
                            tp[:, 128 * qc:128 * qc + 128], osb[qc], IDENT)
                    nc.vector.tensor_copy(OT[hp][:, q0:q0 + 512], tp)

                # ---- emission schedule ------------------------------------
                # feeder queue of work-unit halves pulled into the exp-wait
                # gaps of attention blocks. Each half has an id; a block
                # force-emits any prerequisite halves that the queue has not
                # reached yet, so reads can never precede their writers in
                # trace order.
                from collections import deque
                feeders = deque()
                done = set()
                units = {}

                def def_unit(uid, fns):
                    units[uid] = fns
                    return uid

                for m in range(8):
                    for n in range(4):
                        halves = emit_p2_halves(m, n)
                        def_unit(("p2", m, n, 0), [halves[0]])
                        def_unit(("p2", m, n, 1), [halves[1]])
                for t in range(16):
                    h3 = emit_p3_halves(t)
                    def_unit(("p3", t), h3)
                    def_unit(("p5a", t), [lambda t=t: emit_p5a(t, 0),
                                          lambda t=t: emit_p5a(t, 1)])
                    def_unit(("p5b", t, 0), [lambda t=t: emit_p5b(t, 0)])
                    def_unit(("p5b", t, 1), [lambda t=t: emit_p5b(t, 1)])

                def run(uid):
                    if uid in done:
                        return
                    done.add(uid)
                    for f in units[uid]:
                        f()

                def q2(m, n):
                    feeders.append(("p2", m, n, 0))
                    feeders.append(("p2", m, n, 1))

                def force2(m, n):
                    run(("p2", m, n, 0))
                    run(("p2", m, n, 1))

                def pull_one():
                    while feeders:
                        uid = feeders.popleft()
                        if uid in done:
                            continue
                        run(uid)
                        return

                # prologue: K0 n0 + Q0 n0; the rest feeds into block(0,0)
                force2(NP + 0, 0)
                force2(0, 0)
                for t in range(4):
                    feeders.append(("p3", t))
                for n in range(1, 4):
                    q2(NP + 0, n)
                    for t in range(4 * n, 4 * n + 4):
                        feeders.append(("p3", t))

                def mk_pull(mode, hp, qb):
                    def pull(k):
                        if mode == 2:
                            # first block: its own V/K prerequisites flow in
                            if k < 15:
                                run(("p3", k + 1))
                            if k % 4 == 2 and k < 12:
                                force2(NP, (k + 4) // 4)
                            pull_one()
                        elif mode == 1:
                            if 2 <= k <= 13:
                                pull_one()
                            if k in (4, 8, 12):
                                pull_one()
                        elif 2 <= k <= 11 and k % 2 == 0:
                            pull_one()
                    return pull

                for hp in range(NP):
                    for qb in range(4):
                        if qb < 3:
                            q2(hp, qb + 1)
                        if hp + 1 < NP:
                            if qb == 0:
                                q2(NP + hp + 1, 0)
                                q2(NP + hp + 1, 1)
                            elif qb == 1:
                                q2(NP + hp + 1, 2)
                                q2(NP + hp + 1, 3)
                            elif qb == 2:
                                q2(hp + 1, 0)
                        if hp == 1 and qb > 0:
                            for t in range(4 * (qb - 1), 4 * qb):
                                feeders.append(("p5a", t))
                        if hp == 2 and qb == 0:
                            for t in range(12, 16):
                                feeders.append(("p5a", t))
                        if hp == NP - 1 and qb > 0:
                            for t in range(4 * (qb - 1), 4 * qb):
                                feeders.append(("p5b", t, 0))
                                feeders.append(("p5b", t, 1))
                        # hard prerequisites for this block
                        if hp == 0 and qb == 0:
                            run(("p3", 0))
                        else:
                            for n in range(4):
                                force2(NP + hp, n)      # all K chunks
                            force2(hp, qb)              # this Q chunk
                            for t in range(16):
                                run(("p3", t))
                        emit_block(hp, qb, mk_pull(
                            2 if (hp == 0 and qb == 0) else
                            (1 if hp == NP - 1 else 0), hp, qb))
                while feeders:
                    pull_one()
                for t in range(12, 16):
                    run(("p5b", t, 0))
                    run(("p5b", t, 1))

                if debug_taps:
                    dbg = y_pool.tile([128, T], f32, tag="dbg", name="dbg")
                    for m in range(8):
                        nc.vector.tensor_copy(dbg, QKT[m])
                        nc.sync.dma_start(
                            out=qkt_dbg.ap()[m * 128:(m + 1) * 128, :],
                            in_=dbg)
                    dbg2 = y_pool.tile([128, HL * VW], f32, tag="dbg2",
                                       name="dbg2")
                    for t in range(16):
                        nc.vector.tensor_copy(dbg2, V[t])
                        nc.sync.dma_start(
                            out=v_dbg.ap()[t * 128:(t + 1) * 128, :],
                            in_=dbg2)
                    for p in range(NP):
                        nc.vector.tensor_copy(dbg, OT[p])
                        nc.sync.dma_start(
                            out=ot_dbg.ap()[p * 128:(p + 1) * 128, :],
                            in_=dbg)

    nc.compile()
    return nc


_CACHE = {}


def _get_exec(reps=1):
    """Build + jit the 8-core SPMD executable once per process."""
    key = ("exec", reps)
    if key in _CACHE:
        return _CACHE[key]

    import jax
    from jax.experimental.shard_map import shard_map
    from jax.sharding import Mesh, PartitionSpec
    import concourse.mybir as mybir
    from concourse.bass2jax import (
        _bass_exec_p,
        install_neuronx_cc_hook,
        partition_id_tensor,
    )

    install_neuronx_cc_hook()
    nc = _build_bass(reps)

    partition_name = (
        nc.partition_id_tensor.name if nc.partition_id_tensor else None
    )
    in_names, out_names, out_avals, out_shapes = [], [], [], []
    for alloc in nc.m.functions[0].allocations:
        if not isinstance(alloc, mybir.MemoryLocationSet):
            continue
        name = alloc.memorylocations[0].name
        if alloc.kind == "ExternalInput":
            if name == partition_name:
                continue
            in_names.append(name)
        elif alloc.kind == "ExternalOutput":
            out_names.append(name)
            shape = tuple(alloc.tensor_shape)
            dtype = mybir.dt.np(alloc.dtype)
            out_avals.append(jax.core.ShapedArray(shape, dtype))
            out_shapes.append((shape, dtype))
    n_params = len(in_names)
    n_outs = len(out_names)
    all_names = in_names + out_names
    if partition_name is not None:
        all_names = all_names + [partition_name]

    def _body(*args):
        operands = list(args)
        if partition_name is not None:
            operands.append(partition_id_tensor())
        outs = _bass_exec_p.bind(
            *operands,
            out_avals=tuple(out_avals),
            in_names=tuple(all_names),
            out_names=tuple(out_names),
            lowering_input_output_aliases=(),
            sim_require_finite=True,
            sim_require_nnan=True,
            nc=nc,
        )
        return tuple(outs)

    devices = jax.devices()[:NCORES]
    mesh = Mesh(np.asarray(devices), ("core",))
    donate = tuple(range(n_params, n_params + n_outs))
    sharded = jax.jit(
        shard_map(
            _body,
            mesh=mesh,
            in_specs=(PartitionSpec("core"),) * (n_params + n_outs),
            out_specs=(PartitionSpec("core"),) * n_outs,
            check_rep=False,
        ),
        donate_argnums=donate,
        keep_unused=True,
    )
    _CACHE[key] = (sharded, in_names, out_names, out_shapes)
    return _CACHE[key]


def _shard_inputs(x, w_qkv, w_proj):
    """Per-core input dict, keyed by DRAM tensor name."""
    import ml_dtypes
    bf = ml_dtypes.bfloat16
    x = np.ascontiguousarray(np.asarray(x, dtype=np.float32))
    w_qkv = np.asarray(w_qkv, dtype=np.float32)
    w_proj = np.asarray(w_proj, dtype=np.float32)
    maps = []
    for c in range(NCORES):
        b, g = c // G, c % G
        wq = w_qkv[:, g * F:(g + 1) * F]
        wk = w_qkv[:, C + g * F:C + (g + 1) * F]
        wv = w_qkv[:, 2 * C + g * F:2 * C + (g + 1) * F]
        maps.append({
            "xt": np.ascontiguousarray(x[b].T).astype(bf),
            "wqkv": np.ascontiguousarray(
                np.concatenate([wq, wk, wv], axis=1)).astype(bf),
            "wproj": np.ascontiguousarray(w_proj[g * F:(g + 1) * F, :]).astype(bf),
            "ident": np.eye(128, dtype=np.float32).astype(bf),
        })
    return maps


def _run_cores(in_maps):
    """Execute the SPMD program; returns list of per-core output dicts."""
    sharded, in_names, out_names, out_shapes = _get_exec()
    concat_in = [
        np.concatenate([m[name] for m in in_maps], axis=0) for name in in_names
    ]
    concat_zeros = [
        np.zeros((NCORES * s[0],) + tuple(s[1:]), dt) for s, dt in out_shapes
    ]
    out_arrs = sharded(*concat_in, *concat_zeros)
    outs = []
    for c in range(NCORES):
        outs.append({
            name: np.asarray(out_arrs[i]).reshape((NCORES,) + out_shapes[i][0])[c]
            for i, name in enumerate(out_names)
        })
    return outs


def kernel(x, w_qkv, w_proj, b_proj):
    in_maps = _shard_inputs(x, w_qkv, w_proj)
    outs = _run_cores(in_maps)
    b_proj = np.asarray(b_proj, dtype=np.float32)
    y = np.empty((B, N, C), dtype=np.float32)
    for b in range(B):
        y[b] = outs[G * b]["y"] + outs[G * b + 1]["y"] + b_proj
    return y


if __name__ == "__main__":
    # compile-only sanity check
    import time as _time

    t0 = _time.time()
    nc = _build_bass()
    print(f"bacc build+compile OK ({_time.time()-t0:.1f}s)")


# revision 48
# speedup vs baseline: 1.6920x; 1.6920x over previous
"""Multi-head attention (B=4, N=2048, C=1024, H=16) on 8 Trainium2 NeuronCores.

Sharding: data parallel over batch (4-way) x tensor parallel over heads
(2-way, 8 heads per group). Core c handles batch c//2 and head group c%2.
Each core computes a partial projection output [2048, 1024]; the host sums
the two head-group partials per batch and adds b_proj.

Per-core design (ScalarE exp ~267us and PE ~280us are jointly binding):
  - bf16 on all PE paths (x, weights, Q/K/V, P, O): halves SBUF and DMA.
  - attention in blocks of (head pair hp, 512-query qb): per key chunk k,
    two QK matmuls [64,128]x[64,512] write S^T [128keys,1024] into a
    double-buffered 2-bank PSUM scratch; ONE ScalarE activation(Exp,
    scale) [128,1024] covers both heads (big-FD exp minimizes ACT
    per-instruction overhead, the dominant engine cost).
  - PV with the P chunk as the STATIONARY operand: out O[128 queries,
    65(V..., ones-denominator)] accumulates per 128-query column group, so
    all 128 out partitions are useful (halves PE time vs V-stationary).
    Four 65-col accumulation chains share each PSUM bank; the bank is
    pre-zeroed by one zero-weight matmul so all chains accumulate with
    start=False (interleaved start=True chains corrupt PSUM).
  - normalization: per-partition reciprocal_approx_fast of the ones
    column, one tensor_scalar per (qc,h), PE-transpose (identity matmul)
    back to feature-major OT for the output projection.
  - emission schedule: projection work (Q/K/V proj, output proj halves)
    is queued as small units and pulled into the exp-wait gaps of the
    in-order PE queue;each block force-emits its prerequisites so trace
    order never has a read before its writer. Output proj is split into
    hp0+1 partials (staged bf16 during the hp1 sweep) and hp2+3+add, so
    the final head-pair sweep and tail stay short.
  - optional DVE Schraudolph exp (tensor_scalar fp32->int16 bitcast bf16,
    DVE_KS chunks per block) exists as a tuning knob; currently off.
"""

import numpy as np

B, N, C, H, D = 4, 2048, 1024, 16, 64
SCALE = float(D) ** -0.5
T = 2048          # tokens per core (one batch)
G = 2             # head groups
HL = H // G       # 8 local heads
NP = HL // 2      # 4 head pairs
F = HL * D        # 512 local features
VW = 66           # V tile column stride per head (64 data + 1 ones + 1 pad)
NCORES = 8

LOG2E = 1.4426950408889634
C_CORR = 0.04303566602587917
# DVE-exp chunk pattern within each 16-chunk block (tunable offload ratio)
DVE_KS = ()


def _build_bass(reps=1, dve_ks=DVE_KS, use_gpsimd=True, debug_taps=False):
    import concourse.bacc as bacc
    import concourse.tile as tile
    from concourse import mybir
    import concourse.bass as bass
    f32 = mybir.dt.float32
    bf16 = mybir.dt.bfloat16
    i16 = mybir.dt.int16
    EXP = mybir.ActivationFunctionType.Exp
    MULT = mybir.AluOpType.mult
    ADD = mybir.AluOpType.add

    A_MUL = float(SCALE * LOG2E * (1 << 7))
    B_ADD = float((127.0 - C_CORR) * (1 << 7))

    nc = bacc.Bacc("TRN2", debug=False, num_devices=NCORES)
    xt_d = nc.dram_tensor("xt", [C, T], bf16, kind="ExternalInput")
    wqkv_d = nc.dram_tensor("wqkv", [C, 3 * F], bf16, kind="ExternalInput")
    wproj_d = nc.dram_tensor("wproj", [F, C], bf16, kind="ExternalInput")
    ident_d = nc.dram_tensor("ident", [128, 128], bf16, kind="ExternalInput")
    y_d = nc.dram_tensor("y", [T, C], f32, kind="ExternalOutput")
    if debug_taps:
        qkt_dbg = nc.dram_tensor("qkt_dbg", [8 * 128, T], f32,
                                 kind="ExternalOutput")
        v_dbg = nc.dram_tensor("v_dbg", [16 * 128, HL * VW], f32,
                               kind="ExternalOutput")
        ot_dbg = nc.dram_tensor("ot_dbg", [NP * 128, T], f32,
                                kind="ExternalOutput")

    with tile.TileContext(nc) as tc:
        with (
            tc.tile_pool(name="xT", bufs=1) as xT_pool,
            tc.tile_pool(name="wv", bufs=1) as wv_pool,
            tc.tile_pool(name="wqk", bufs=1) as wqk_pool,
            tc.tile_pool(name="wp", bufs=1) as wp_pool,
            tc.tile_pool(name="qkt", bufs=1) as qkt_pool,
            tc.tile_pool(name="vsb", bufs=1) as v_pool,
            tc.tile_pool(name="ot", bufs=1) as ot_pool,
            tc.tile_pool(name="ptile", bufs=7) as p_pool,
            tc.tile_pool(name="ocp", bufs=6) as ocp_pool,
            tc.tile_pool(name="nrm", bufs=2) as nrm_pool,
            tc.tile_pool(name="ysb", bufs=2) as y_pool,
            tc.tile_pool(name="y01p", bufs=16) as y01_pool,
            tc.tile_pool(name="consts", bufs=1) as consts,
            tc.tile_pool(name="psqk", bufs=2, space="PSUM") as ps_qk,
            tc.tile_pool(name="psot", bufs=1, space="PSUM") as ps_ot,
            tc.tile_pool(name="psproj", bufs=2, space="PSUM") as ps_proj,
        ):
            IDENT = consts.tile([128, 128], bf16, name="IDENT")
            nc.sync.dma_start(out=IDENT, in_=ident_d.ap())
            ZW = consts.tile([128, 128], bf16, name="ZW")
            nc.vector.memset(ZW, 0.0)
            for _rep in range(reps):
                XT = [xT_pool.tile([128, T], bf16, tag=f"xT{c}", name=f"xT{c}")
                      for c in range(8)]
                WV = [wv_pool.tile([128, F], bf16, tag=f"wv{c}", name=f"wv{c}")
                      for c in range(8)]
                WQK = [wqk_pool.tile([128, 2 * F], bf16, tag=f"wqk{c}",
                                     name=f"wqk{c}") for c in range(8)]
                WP = [wp_pool.tile([128, C], bf16, tag=f"wp{p}", name=f"wp{p}")
                      for p in range(NP)]
                QKT = [qkt_pool.tile([128, T], bf16, tag=f"qkt{m}",
                                     name=f"qkt{m}") for m in range(8)]
                V = [v_pool.tile([128, HL * VW], bf16, tag=f"v{t}",
                                 name=f"v{t}") for t in range(16)]
                OT = [ot_pool.tile([128, T], bf16, tag=f"ot{p}", name=f"ot{p}")
                      for p in range(NP)]

                # loads: WQK/XT interleaved (HWDGE, prologue-critical);
                # WV/WP on the idle gpsimd SWDGE path
                for c in range(8):
                    nc.sync.dma_start(
                        out=WQK[c],
                        in_=wqkv_d.ap()[c * 128:(c + 1) * 128, 0:2 * F])
                    nc.sync.dma_start(out=XT[c],
                                      in_=xt_d.ap()[c * 128:(c + 1) * 128, :])
                for c in range(8):
                    nc.sync.dma_start(
                        out=WV[c],
                        in_=wqkv_d.ap()[c * 128:(c + 1) * 128, 2 * F:3 * F])
                for p in range(NP):
                    nc.sync.dma_start(
                        out=WP[p],
                        in_=wproj_d.ap()[p * 128:(p + 1) * 128, :])

                # ---- projection work units (emitted interleaved) ----------
                def emit_p2_halves(m, n):
                    """Q/K proj for QKT[m] tokens [512n, 512n+512) as two
                    pullable half-units (4 contraction chunks each)."""
                    qp = ps_proj.tile([128, 512], f32, tag="proj", name="qp")

                    def half(lo):
                        for c in range(lo, lo + 4):
                            nc.tensor.matmul(
                                qp,
                                WQK[c][:, m * 128:(m + 1) * 128],
                                XT[c][:, n * 512:(n + 1) * 512],
                                start=(c == 0), stop=(c == 7),
                            )
                        if lo == 4:
                            nc.vector.tensor_copy(
                                QKT[m][:, n * 512:(n + 1) * 512], qp)
                    return [lambda: half(0), lambda: half(4)]

                def emit_p2(m, n):
                    for u in emit_p2_halves(m, n):
                        u()

                def emit_p3_halves(t):
                    """V proj for token chunk t as two pullable halves."""
                    vp = ps_proj.tile([128, F], f32, tag="proj", name="vp")

                    def half(lo):
                        for c in range(lo, lo + 4):
                            nc.tensor.matmul(
                                vp,
                                XT[c][:, t * 128:(t + 1) * 128],
                                WV[c],
                                start=(c == 0), stop=(c == 7),
                            )
                        if lo == 4:
                            v3 = V[t].rearrange("p (h w) -> p h w", w=VW)
                            nc.vector.memset(v3[:, :, 64:65], 1.0)
                            nc.vector.tensor_copy(
                                v3[:, :, 0:64],
                                vp.rearrange("p (h w) -> p h w", w=64))
                    return [lambda: half(0), lambda: half(4)]

                def emit_p5a(t, n):
                    """output proj partial hp0+hp1 -> bf16 staging."""
                    yp = ps_proj.tile([128, 512], f32, tag="proj", name="yp")
                    for hp in range(2):
                        nc.tensor.matmul(
                            yp,
                            OT[hp][:, t * 128:(t + 1) * 128],
                            WP[hp][:, n * 512:(n + 1) * 512],
                            start=(hp == 0), stop=(hp == 1),
                        )
                    if n == 0:
                        y01s[t] = y01_pool.tile([128, 1024], bf16,
                                                tag="y01", name="y01")
                    nc.vector.tensor_copy(
                        y01s[t][:, n * 512:(n + 1) * 512], yp)

                def emit_p5b(t, n):
                    """hp2+hp3 + staged partial -> y DMA (merged n-halves)."""
                    yp = ps_proj.tile([128, 512], f32, tag="proj", name="yp")
                    for hp in range(2, 4):
                        nc.tensor.matmul(
                            yp,
                            OT[hp][:, t * 128:(t + 1) * 128],
                            WP[hp][:, n * 512:(n + 1) * 512],
                            start=(hp == 2), stop=(hp == 3),
                        )
                    if n == 0:
                        yts[t] = y_pool.tile([128, 1024], f32, tag="yt",
                                             name="yt")
                    nc.vector.tensor_add(
                        yts[t][:, n * 512:(n + 1) * 512], yp,
                        y01s[t][:, n * 512:(n + 1) * 512])
                    if n == 1:
                        nc.sync.dma_start(
                            out=y_d.ap()[t * 128:(t + 1) * 128, :],
                            in_=yts[t])

                yts = [None] * 16
                y01s = [None] * 16

                # ---- attention block: head pair hp, query block qb --------
                # O accumulates query-major: bank j holds qc=2j,2j+1, each
                # [128q, 65(h0)|65(h1)] -> [128, 260]; then per-partition
                # reciprocal of the ones-column, tensor_scalar normalize to
                # bf16, PE-transpose back to feature-major OT.
                def emit_block(hp, qb, pull):
                    q0 = qb * 512
                    KT = QKT[NP + hp]
                    QT = QKT[hp]
                    O = [ps_ot.tile([128, 260], f32, tag=f"o{j}", name=f"o{j}")
                         for j in range(2)]

                    def emit_zinit():
                        for j in range(2):
                            nc.tensor.matmul(
                                O[j], ZW, QKT[0][:, 0:260],
                                start=True, stop=False, skip_group_check=True)

                    def emit_pv(k, P):
                        for h in range(2):
                            lh = 2 * hp + h
                            for qc in range(4):
                                nc.tensor.matmul(
                                    O[qc // 2][:, 130 * (qc % 2) + 65 * h:
                                               130 * (qc % 2) + 65 * h + 65],
                                    P[:, 512 * h + 128 * qc:
                                      512 * h + 128 * qc + 128],
                                    V[k][:, lh * VW:lh * VW + 65],
                                    start=False, stop=(k == 15),
                                    skip_group_check=True,
                                )

                    pvq = []
                    for k in range(16):
                        s = ps_qk.tile([128, 1024], f32, tag="s", name="s")
                        for h in range(2):
                            nc.tensor.matmul(
                                s[:, 512 * h:512 * h + 512],
                                KT[64 * h:64 * h + 64,
                                   k * 128:(k + 1) * 128],
                                QT[64 * h:64 * h + 64, q0:q0 + 512],
                                start=True, stop=True,
                            )
                        P = p_pool.tile([128, 1024], bf16, tag="P", name="P")
                        if k in dve_ks:
                            nc.vector.tensor_scalar(
                                out=P.bitcast(i16), in0=s,
                                scalar1=A_MUL, scalar2=B_ADD,
                                op0=MULT, op1=ADD)
                        else:
                            nc.scalar.activation(P, s, EXP, scale=SCALE)
                        if k == 0:
                            emit_zinit()
                        pvq.append((k, P))
                        if len(pvq) > 4:
                            kk, PP = pvq.pop(0)
                            emit_pv(kk, PP)
                        pull(k)
                    for kk, PP in pvq:
                        emit_pv(kk, PP)
                    osb = [None] * 4
                    for j in range(2):
                        dens = O[j].rearrange("p (g w) -> p g w", w=65)
                        dcp = nrm_pool.tile([128, 4], f32, tag="dcp",
                                            name="dcp")
                        nc.vector.tensor_copy(
                            dcp.rearrange("p (g w) -> p g w", w=1),
                            dens[:, :, 64:65])
                        rec = nrm_pool.tile([128, 4], f32, tag="rec",
                                            name="rec")
                        nc.vector.reciprocal_approx_fast(rec, dcp)
                        for c in range(2):
                            qc = 2 * j + c
                            ob = ocp_pool.tile([128, 128], bf16, tag="osb",
                                               name="osb")
                            for h in range(2):
                                nc.vector.tensor_scalar(
                                    out=ob[:, 64 * h:64 * h + 64],
                                    in0=O[j][:, 130 * c + 65 * h:
                                             130 * c + 65 * h + 64],
                                    scalar1=rec[:, 2 * c + h:2 * c + h + 1],
                                    scalar2=None, op0=MULT)
                            osb[qc] = ob
                    tp = ps_ot.tile([128, 512], bf16, tag="o0", name="tp")
                    for qc in range(4):
                        nc.tensor.transpose(
                            tp[:, 128 * qc:128 * qc + 128], osb[qc], IDENT)
                    nc.vector.tensor_copy(OT[hp][:, q0:q0 + 512], tp)

                # ---- emission schedule ------------------------------------
                # feeder queue of work-unit halves pulled into the exp-wait
                # gaps of attention blocks. Each half has an id; a block
                # force-emits any prerequisite halves that the queue has not
                # reached yet, so reads can never precede their writers in
                # trace order.
                from collections import deque
                feeders = deque()
                done = set()
                units = {}

                def def_unit(uid, fns):
                    units[uid] = fns
                    return uid

                for m in range(8):
                    for n in range(4):
                        halves = emit_p2_halves(m, n)
                        def_unit(("p2", m, n, 0), [halves[0]])
                        def_unit(("p2", m, n, 1), [halves[1]])
                for t in range(16):
                    h3 = emit_p3_halves(t)
                    def_unit(("p3", t), h3)
                    def_unit(("p5a", t), [lambda t=t: emit_p5a(t, 0),
                                          lambda t=t: emit_p5a(t, 1)])
                    def_unit(("p5b", t, 0), [lambda t=t: emit_p5b(t, 0)])
                    def_unit(("p5b", t, 1), [lambda t=t: emit_p5b(t, 1)])

                def run(uid):
                    if uid in done:
                        return
                    done.add(uid)
                    for f in units[uid]:
                        f()

                def q2(m, n):
                    feeders.append(("p2", m, n, 0))
                    feeders.append(("p2", m, n, 1))

                def force2(m, n):
                    run(("p2", m, n, 0))
                    run(("p2", m, n, 1))

                def pull_one():
                    while feeders:
                        uid = feeders.popleft()
                        if uid in done:
                            continue
                        run(uid)
                        return

                # prologue: K0 n0 + Q0 n0; the rest feeds into block(0,0)
                force2(NP + 0, 0)
                force2(0, 0)
                for t in range(4):
                    feeders.append(("p3", t))
                for n in range(1, 4):
                    q2(NP + 0, n)
                    for t in range(4 * n, 4 * n + 4):
                        feeders.append(("p3", t))

                def mk_pull(mode, hp, qb):
                    def pull(k):
                        if mode == 2:
                            # first block: its own V/K prerequisites flow in
                            if k < 15:
                                run(("p3", k + 1))
                            if k % 4 == 2 and k < 12:
                                force2(NP, (k + 4) // 4)
                        elif mode == 1:
                            if 2 <= k <= 13:
                                pull_one()
                            if k in (4, 8, 12):
                                pull_one()
                        elif 2 <= k <= 11 and k % 2 == 0:
                            pull_one()
                    return pull

                for hp in range(NP):
                    for qb in range(4):
                        if qb < 3:
                            q2(hp, qb + 1)
                        if hp + 1 < NP:
                            if qb == 0:
                                q2(NP + hp + 1, 0)
                                q2(NP + hp + 1, 1)
                            elif qb == 1:
                                q2(NP + hp + 1, 2)
                                q2(NP + hp + 1, 3)
                            elif qb == 2:
                                q2(hp + 1, 0)
                        if hp == 1 and qb > 0:
                            for t in range(4 * (qb - 1), 4 * qb):
                                feeders.append(("p5a", t))
                        if hp == 2 and qb == 0:
                            for t in range(12, 16):
                                feeders.append(("p5a", t))
                        if hp == NP - 1 and qb > 0:
                            for t in range(4 * (qb - 1), 4 * qb):
                                feeders.append(("p5b", t, 0))
                                feeders.append(("p5b", t, 1))
                        # hard prerequisites for this block
                        if hp == 0 and qb == 0:
                            run(("p3", 0))
                        else:
                            for n in range(4):
                                force2(NP + hp, n)      # all K chunks
                            force2(hp, qb)              # this Q chunk
                            for t in range(16):
                                run(("p3", t))
                        emit_block(hp, qb, mk_pull(
                            2 if (hp == 0 and qb == 0) else
                            (1 if hp == NP - 1 else 0), hp, qb))
                while feeders:
                    pull_one()
                for t in range(12, 16):
                    run(("p5b", t, 0))
                    run(("p5b", t, 1))

                if debug_taps:
                    dbg = y_pool.tile([128, T], f32, tag="dbg", name="dbg")
                    for m in range(8):
                        nc.vector.tensor_copy(dbg, QKT[m])
                        nc.sync.dma_start(
                            out=qkt_dbg.ap()[m * 128:(m + 1) * 128, :],
                            in_=dbg)
                    dbg2 = y_pool.tile([128, HL * VW], f32, tag="dbg2",
                                       name="dbg2")
                    for t in range(16):
                        nc.vector.tensor_copy(dbg2, V[t])
                        nc.sync.dma_start(
                            out=v_dbg.ap()[t * 128:(t + 1) * 128, :],
                            in_=dbg2)
                    for p in range(NP):
                        nc.vector.tensor_copy(dbg, OT[p])
                        nc.sync.dma_start(
                            out=ot_dbg.ap()[p * 128:(p + 1) * 128, :],
                            in_=dbg)

    nc.compile()
    return nc


_CACHE = {}


def _get_exec(reps=1):
    """Build + jit the 8-core SPMD executable once per process."""
    key = ("exec", reps)
    if key in _CACHE:
        return _CACHE[key]

    import jax
    from jax.experimental.shard_map import shard_map
    from jax.sharding import Mesh, PartitionSpec
    import concourse.mybir as mybir
    from concourse.bass2jax import (
        _bass_exec_p,
        install_neuronx_cc_hook,
        partition_id_tensor,
    )

    install_neuronx_cc_hook()
    nc = _build_bass(reps)

    partition_name = (
        nc.partition_id_tensor.name if nc.partition_id_tensor else None
    )
    in_names, out_names, out_avals, out_shapes = [], [], [], []
    for alloc in nc.m.functions[0].allocations:
        if not isinstance(alloc, mybir.MemoryLocationSet):
            continue
        name = alloc.memorylocations[0].name
        if alloc.kind == "ExternalInput":
            if name == partition_name:
                continue
            in_names.append(name)
        elif alloc.kind == "ExternalOutput":
            out_names.append(name)
            shape = tuple(alloc.tensor_shape)
            dtype = mybir.dt.np(alloc.dtype)
            out_avals.append(jax.core.ShapedArray(shape, dtype))
            out_shapes.append((shape, dtype))
    n_params = len(in_names)
    n_outs = len(out_names)
    all_names = in_names + out_names
    if partition_name is not None:
        all_names = all_names + [partition_name]

    def _body(*args):
        operands = list(args)
        if partition_name is not None:
            operands.append(partition_id_tensor())
        outs = _bass_exec_p.bind(
            *operands,
            out_avals=tuple(out_avals),
            in_names=tuple(all_names),
            out_names=tuple(out_names),
            lowering_input_output_aliases=(),
            sim_require_finite=True,
            sim_require_nnan=True,
            nc=nc,
        )
        return tuple(outs)

    devices = jax.devices()[:NCORES]
    mesh = Mesh(np.asarray(devices), ("core",))
    donate = tuple(range(n_params, n_params + n_outs))
    sharded = jax.jit(
        shard_map(
            _body,
            mesh=mesh,
            in_specs=(PartitionSpec("core"),) * (n_params + n_outs),
            out_specs=(PartitionSpec("core"),) * n_outs,
            check_rep=False,
        ),
        donate_argnums=donate,
        keep_unused=True,
    )
    _CACHE[key] = (sharded, in_names, out_names, out_shapes)
    return _CACHE[key]


def _shard_inputs(x, w_qkv, w_proj):
    """Per-core input dict, keyed by DRAM tensor name."""
    import ml_dtypes
    bf = ml_dtypes.bfloat16
    x = np.ascontiguousarray(np.asarray(x, dtype=np.float32))
    w_qkv = np.asarray(w_qkv, dtype=np.float32)
    w_proj = np.asarray(w_proj, dtype=np.float32)
    maps = []
    for c in range(NCORES):
        b, g = c // G, c % G
        wq = w_qkv[:, g * F:(g + 1) * F]
        wk = w_qkv[:, C + g * F:C + (g + 1) * F]
        wv = w_qkv[:, 2 * C + g * F:2 * C + (g + 1) * F]
        maps.append({
            "xt": np.ascontiguousarray(x[b].T).astype(bf),
            "wqkv": np.ascontiguousarray(
                np.concatenate([wq, wk, wv], axis=1)).astype(bf),
            "wproj": np.ascontiguousarray(w_proj[g * F:(g + 1) * F, :]).astype(bf),
            "ident": np.eye(128, dtype=np.float32).astype(bf),
        })
    return maps


def _run_cores(in_maps):
    """Execute the SPMD program; returns list of per-core output dicts."""
    sharded, in_names, out_names, out_shapes = _get_exec()
    concat_in = [
        np.concatenate([m[name] for m in in_maps], axis=0) for name in in_names
    ]
    concat_zeros = [
        np.zeros((NCORES * s[0],) + tuple(s[1:]), dt) for s, dt in out_shapes
    ]
    out_arrs = sharded(*concat_in, *concat_zeros)
    outs = []
    for c in range(NCORES):
        outs.append({
            name: np.asarray(out_arrs[i]).reshape((NCORES,) + out_shapes[i][0])[c]
            for i, name in enumerate(out_names)
        })
    return outs


def kernel(x, w_qkv, w_proj, b_proj):
    in_maps = _shard_inputs(x, w_qkv, w_proj)
    outs = _run_cores(in_maps)
    b_proj = np.asarray(b_proj, dtype=np.float32)
    y = np.empty((B, N, C), dtype=np.float32)
    for b in range(B):
        y[b] = outs[G * b]["y"] + outs[G * b + 1]["y"] + b_proj
    return y


if __name__ == "__main__":
    # compile-only sanity check
    import time as _time

    t0 = _time.time()
    nc = _build_bass()
    print(f"bacc build+compile OK ({_time.time()-t0:.1f}s)")
